# revision 1
# baseline (speedup 1.0000x reference)
"""
Echo-State-Network (HDESN) reservoir kernel for Trainium2 (Bass/Tile).

Reference computation (T=4096, DIMS=64, RESERVOIR=2048):
    U = (C @ x_t)            for all t            -> (T, 64)
    h_t = tanh(W_in u_t + W_res h_{t-1})          (sequential recurrence)
    y_t = dense_W @ [u_t; h_t] + dense_b          -> (T, 1)

Strategy (single-core sequential recurrence; the 4096-step tanh
recurrence cannot be parallelized across cores without a per-step
collective whose ~7-20us latency floor dwarfs the ~14us/step compute):

  * W_res^T is SBUF-resident in bf16; each step does 16x16 (K=128,M=128)
    stationary-weight matmuls with the h vector as the N=1 moving operand.
    bf16 weights enable Fast Weight Load; output lands column-major in
    PSUM which feeds ScalarE tanh directly and yields next step's rhs.
  * The dense output row w_h rides along as an extra M=1 stationary
    column per step producing y_raw[t] = w_h . h_{t-1} for free;
    y[t] = y_raw[t+1] + (U @ w_u)[t] + b is assembled at the end.
  * B = W_in @ U^T is precomputed on-device per 512-step block.
  * All 8 cores run the same program (SPMD); core 0's output is used.
"""

import sys
import os

sys.path.insert(0, "/opt/trn_rl_repo")

import numpy as np
import ml_dtypes

T = 4096
DIMS = 64
RES = 2048
KT = RES // 128          # 16 k-tiles
MT = RES // 128          # 16 m-tiles
BLK = 512                # steps per block (B-precompute granularity)
N_CORES = 8

_F32 = None
_BF16 = None


def _build(t_steps: int, unroll: int = 2):
    """Trace the bass program. Returns (nc, input_names)."""
    import concourse.bass as bass
    import concourse.bacc as bacc
    import concourse.tile as tile
    from concourse import mybir

    global _F32, _BF16
    f32 = mybir.dt.float32
    bf16 = mybir.dt.bfloat16
    _F32, _BF16 = f32, bf16
    AF = mybir.ActivationFunctionType

    assert t_steps % BLK == 0
    n_blocks = t_steps // BLK

    nc = bacc.Bacc("TRN2", target_bir_lowering=False, debug=False,
                   num_devices=N_CORES)

    # ---- external I/O ----------------------------------------------------
    # WT[p, k, m-flat]: WT[p, k, j] = W_res[j, k*128 + p]  (lhsT layout)
    WT_d = nc.dram_tensor("WT", [128, KT, RES], bf16, kind="ExternalInput").ap()
    # whT[p, k] = dense_W[0, 64 + k*128 + p]
    whT_d = nc.dram_tensor("whT", [128, KT], bf16, kind="ExternalInput").ap()
    # WinT[d, r] = W_in[r, d]
    WinT_d = nc.dram_tensor("WinT", [DIMS, RES], f32, kind="ExternalInput").ap()
    # CT[i, d] = C[d, i]
    CT_d = nc.dram_tensor("CT", [DIMS, DIMS], f32, kind="ExternalInput").ap()
    # XT[i, t] = X[t, i, 0]
    XT_d = nc.dram_tensor("XT", [DIMS, T], f32, kind="ExternalInput").ap()
    # wu[d, 0] = dense_W[0, d]
    wu_d = nc.dram_tensor("wu", [DIMS, 1], f32, kind="ExternalInput").ap()
    bias_d = nc.dram_tensor("bias", [1, 1], f32, kind="ExternalInput").ap()
    Y_d = nc.dram_tensor("Y", [1, t_steps], f32, kind="ExternalOutput").ap()

    with tile.TileContext(nc) as tc:
        from contextlib import ExitStack
        ctx = ExitStack()
        consts = ctx.enter_context(tc.tile_pool(name="consts", bufs=1))
        work = ctx.enter_context(tc.tile_pool(name="work", bufs=2))
        psum_pool = ctx.enter_context(
            tc.tile_pool(name="psum", bufs=2, space="PSUM"))
        psum_big = ctx.enter_context(
            tc.tile_pool(name="psum_big", bufs=2, space="PSUM"))

        # ---- load constants ---------------------------------------------
        WT = consts.tile([128, KT, RES], bf16)
        nc.sync.dma_start(WT[:], WT_d[:])
        whT = consts.tile([128, KT], bf16)
        nc.sync.dma_start(whT[:], whT_d[:])
        WinT = consts.tile([DIMS, RES], f32)
        nc.sync.dma_start(WinT[:], WinT_d[:])
        CT = consts.tile([DIMS, DIMS], f32)
        nc.sync.dma_start(CT[:], CT_d[:])
        XT = consts.tile([DIMS, T], f32)
        nc.sync.dma_start(XT[:], XT_d[:])
        wu = consts.tile([DIMS, 1], f32)
        nc.sync.dma_start(wu[:], wu_d[:])
        bias = consts.tile([1, 1], f32)
        nc.sync.dma_start(bias[:], bias_d[:])

        # ---- U^T = C^T.T @ X^T  (64 x T) --------------------------------
        UT = consts.tile([DIMS, T], f32)
        for n in range(T // 512):
            pu = psum_big.tile([DIMS, 512], f32, tag="pbig")
            nc.tensor.matmul(pu[:], CT[:], XT[:, n * 512:(n + 1) * 512],
                             start=True, stop=True)
            nc.vector.tensor_copy(UT[:, n * 512:(n + 1) * 512], pu[:])

        # persistent step-state tiles
        hA = consts.tile([128, KT], bf16)
        hB = consts.tile([128, KT], bf16)
        nc.vector.memset(hA[:], 0.0)
        y_raw = consts.tile([1, t_steps + 1], f32)
        Baug = consts.tile([1, t_steps], f32)
        B_sb = consts.tile([128, MT, BLK], f32)

        def reservoir_step(h_in, h_out, p_main, p_aug, tmp, y_idx, b_idx):
            """One recurrence step: h_out = tanh(W h_in + B[:, b_idx]);
            y_raw[y_idx] = w_h . h_in."""
            for m in range(MT):
                for k in range(KT):
                    nc.tensor.matmul(
                        p_main[:, m:m + 1],
                        WT[:, k, m * 128:(m + 1) * 128],
                        h_in[:, k:k + 1],
                        start=(k == 0), stop=(k == KT - 1))
            for k in range(KT):
                nc.tensor.matmul(
                    p_aug[0:1, 0:1], whT[:, k:k + 1], h_in[:, k:k + 1],
                    start=(k == 0), stop=(k == KT - 1))
            nc.vector.tensor_add(tmp[:], p_main[:], B_sb[:, :, b_idx])
            nc.scalar.activation(h_out[:], tmp[:], AF.Tanh)
            nc.vector.tensor_copy(y_raw[0:1, y_idx], p_aug[0:1, 0:1])

        for b in range(n_blocks):
            # ---- B block: B_sb[:, m, s] = (W_in @ u_{b*BLK+s})[m*128:+128]
            # and Baug[0, b*BLK+s] = w_u . u_{b*BLK+s} + bias
            rhsU = UT[:, b * BLK:(b + 1) * BLK]
            for m in range(MT):
                pb = psum_big.tile([128, BLK], f32, tag="pbig")
                nc.tensor.matmul(pb[:], WinT[:, m * 128:(m + 1) * 128], rhsU,
                                 start=True, stop=True)
                nc.vector.tensor_copy(B_sb[:, m, :], pb[:])
            pba = psum_big.tile([1, BLK], f32, tag="pbig")
            nc.tensor.matmul(pba[:], wu[:], rhsU, start=True, stop=True)
            nc.scalar.activation(Baug[0:1, b * BLK:(b + 1) * BLK], pba[:],
                                 AF.Identity, bias=bias[0:1, 0:1])

            # ---- the sequential steps ------------------------------------
            n_iters = BLK // unroll
            with tc.For_i(0, n_iters, 1,
                          hint_engines=(mybir.EngineType.PE,)) as i:
                for j in range(unroll):
                    h_in, h_out = (hA, hB) if j % 2 == 0 else (hB, hA)
                    p_main = psum_pool.tile([128, MT], f32, tag="pm")
                    p_aug = psum_pool.tile([1, 1], f32, tag="pa")
                    tmp = work.tile([128, MT], f32, tag="tmp")
                    step_off = b * BLK + j
                    reservoir_step(
                        h_in, h_out, p_main, p_aug, tmp,
                        y_idx=bass.ds(i * unroll + step_off, 1),
                        b_idx=bass.ds(i * unroll + j, 1))

        # extra step t_steps: only y_raw[t_steps] = w_h . h_{t_steps-1}
        h_last = hA if (t_steps % 2 == 0) else hB
        p_aug = psum_pool.tile([1, 1], f32, tag="pa")
        for k in range(KT):
            nc.tensor.matmul(p_aug[0:1, 0:1], whT[:, k:k + 1],
                             h_last[:, k:k + 1],
                             start=(k == 0), stop=(k == KT - 1))
        nc.vector.tensor_copy(y_raw[0:1, t_steps:t_steps + 1],
                              p_aug[0:1, 0:1])

        # ---- final: y[t] = y_raw[t+1] + Baug[t]  ------------------------
        y_out = consts.tile([1, t_steps], f32)
        nc.vector.tensor_add(y_out[:], y_raw[0:1, 1:t_steps + 1], Baug[:])
        nc.sync.dma_start(Y_d[:], y_out[:])
        ctx.close()

    nc.compile()
    return nc


def _marshal(X, C, W_in, W_res, dense_W, dense_b):
    """Host-side input marshalling into the device layouts."""
    bf = ml_dtypes.bfloat16
    WT = np.ascontiguousarray(
        W_res.T.astype(np.float32).reshape(KT, 128, RES).transpose(1, 0, 2)
    ).astype(bf)                                     # (128, KT, RES)
    w_h = dense_W[0, DIMS:].astype(np.float32)
    whT = np.ascontiguousarray(w_h.reshape(KT, 128).T).astype(bf)  # (128, KT)
    WinT = np.ascontiguousarray(W_in.T).astype(np.float32)         # (64, RES)
    CT = np.ascontiguousarray(C.T).astype(np.float32)
    XT = np.ascontiguousarray(X[:, :, 0].T).astype(np.float32)     # (64, T)
    wu = np.ascontiguousarray(dense_W[0, :DIMS].reshape(DIMS, 1)).astype(
        np.float32)
    bias = np.array([[np.float32(dense_b[0])]], dtype=np.float32)
    return {"WT": WT, "whT": whT, "WinT": WinT, "CT": CT, "XT": XT,
            "wu": wu, "bias": bias}


_CACHED = {}


def run(inputs_np, t_steps=T, unroll=2, trace=False):
    """Build (cached), run on 8 cores, return (y (t_steps,1,1), results)."""
    from concourse.bass_utils import run_bass_kernel_spmd

    key = (t_steps, unroll)
    if key not in _CACHED:
        _CACHED[key] = _build(t_steps, unroll)
    nc = _CACHED[key]
    in_map = _marshal(**inputs_np)
    core_ids = list(range(N_CORES))
    res = run_bass_kernel_spmd(nc, [dict(in_map) for _ in core_ids], core_ids,
                               trace=trace)
    y = np.asarray(res.results[0]["Y"], dtype=np.float32).reshape(t_steps, 1, 1)
    return y, res


def kernel(X, C, W_in, W_res, dense_W, dense_b):
    y, _ = run(dict(X=X, C=C, W_in=W_in, W_res=W_res,
                    dense_W=dense_W, dense_b=dense_b))
    return y

